# revision 1
# baseline (speedup 1.0000x reference)
"""Trainium2 Bass/Tile kernel for nn_DecLayer (GNN message-passing decoder layer).

Math (per node n over K=48 neighbors):
    h_EV    = concat([h_V[n] bcast over K, h_E[n]])            [K, 512]
    m3      = gelu(gelu(h_EV @ W1.T) @ W2.T)  (W3 deferred)    [K, 128]
    dh      = (sum_k mask[k] * m3[k]) @ (W3.T/30)              [128]
    h       = LN1(h_V + dh); h = LN2(h + FFN(h)); out = mask_V * h

Strategy: data-parallel over the B*N = 8192 nodes across 8 cores (1024 each).
The host pre-packs h_E *block-transposed* (feature-major) so every device DMA
is a large contiguous read with features on SBUF partitions. All matmuls keep
features on partitions (weights stationary), so biases are per-partition ACT
bias vectors. The masked K-sum is a DVE grouped reduce. The W3 matmul, both
LayerNorms and the FFN run over [128, nodes] at the end (tiny vs. phase 1).
"""

import numpy as np
from contextlib import ExitStack

import concourse.bass as bass
import concourse.tile as tile
from concourse import mybir
from concourse.bass_utils import run_bass_kernel_spmd
from concourse.vector_clock import ScopedClock

# Problem shapes (fixed by the harness).
B, N, K = 4, 2048, 48
H, HIN, DFF = 128, 384, 512
SCALE, EPS = 30.0, 1e-5
N_CORES = 8
NODES = B * N
NODES_C = NODES // N_CORES   # 1024 nodes per core
TILE_N = 16                  # nodes per phase-1 tile
ROWS = TILE_N * K            # 768 (rows per tile)
HALF = ROWS // 2             # 384 (fp32 matmul free-dim limit is 512)
FP32 = mybir.dt.float32
FP32R = mybir.dt.float32r


def _r(ap):
    """fp32 -> float32r view: PE runs float32r at 1 cycle/row (vs 4 for fp32)
    when the moving dim is >=256, at slightly reduced mantissa precision."""
    return ap.bitcast(FP32R)
AF = mybir.ActivationFunctionType
OP = mybir.AluOpType

_MAX_DRAIN_WAITS = 1  # walrus CTRL codegen accepts only 1 sync wait per Drain


def _patch_tile_drain():
    """Split the Tile tail-drain's sem waits across several Drain insts.

    The stock `_drain_and_barrier` puts every outstanding sem wait on one
    Drain; walrus's CTRL lowering in this toolchain rejects >1 wait.
    """
    if getattr(tile.TileContext, "_drain_patched", False):
        return

    def _drain_and_barrier(self, tick_clock, wait_clock):
        nc = self.nc
        drain_inst = nc.sync.drain()
        wait_clock.add_sem_waits(
            drain_inst.ins, ScopedClock({None: tick_clock.global_clock})
        )
        si = drain_inst.ins.sync_info
        waits = list(si.on_wait or []) if si is not None else []
        if len(waits) > _MAX_DRAIN_WAITS:
            si.on_wait = waits[:_MAX_DRAIN_WAITS]
            rest = waits[_MAX_DRAIN_WAITS:]
            while rest:
                d2 = nc.sync.drain()
                d2.ins.sync_info = mybir.SyncInfo(
                    on_wait=rest[:_MAX_DRAIN_WAITS], on_update=[]
                )
                rest = rest[_MAX_DRAIN_WAITS:]
        nc.all_engine_barrier()
        assert self.sems is not None
        popped = nc._tile_sem_poison_stack.pop()
        assert popped is self._sem_poison
        nc.clear_and_free_semaphores(list(self.sems.allocated().values()))
        nc.all_engine_barrier()

    tile.TileContext._drain_and_barrier = _drain_and_barrier
    tile.TileContext._drain_patched = True


def _split_sync_waits(nc, max_waits=_MAX_DRAIN_WAITS):
    """Hoist excess per-instruction sem waits onto same-engine NOPs.

    This walrus build rejects >1 sync wait on any instruction; a NOP that
    waits immediately before the real instruction is equivalent (same
    engine, program order).
    """
    for f in nc.m.functions:
        for b in f.blocks:
            new_insts = []
            for inst in b.instructions:
                si = getattr(inst, "sync_info", None)
                waits = list(si.on_wait) if si is not None and si.on_wait else []
                if len(waits) > max_waits:
                    head, keep = waits[:-max_waits], waits[-max_waits:]
                    for i in range(0, len(head), max_waits):
                        new_insts.append(
                            mybir.InstNoOp(
                                name=f"{inst.name}w{i}",
                                engine=inst.engine,
                                sync_info=mybir.SyncInfo(
                                    on_wait=head[i : i + max_waits], on_update=[]
                                ),
                                bass_nofuse=True,
                            )
                        )
                    si.on_wait = keep
                new_insts.append(inst)
            b.instructions[:] = new_insts


def build_program(nodes_c=NODES_C, num_devices=N_CORES, b3_nonzero=False,
                  split_waits=True, reps=1):
    """Build the per-core Bass program (SPMD: same program, per-core data)."""
    _patch_tile_drain()
    n_tiles = nodes_c // TILE_N
    n_half = min(512, nodes_c)          # phase-2 chunk width
    n_strips = nodes_c // n_half
    tiles_per_strip = n_half // TILE_N

    nc = bass.Bass(
        "TRN2",
        target_bir_lowering=False,
        debug=False,
        enable_asserts=False,
        num_devices=num_devices,
    )

    dt = nc.dram_tensor
    hE_t = dt("hE_t", [n_tiles, 3, 128, ROWS], FP32, kind="ExternalInput")
    hVT_d = dt("hVT", [128, nodes_c], FP32, kind="ExternalInput")
    mA_d = dt("mask_a", [n_tiles, ROWS], FP32, kind="ExternalInput")
    mV_d = dt("mask_v", [1, nodes_c], FP32, kind="ExternalInput")
    W1T_d = dt("W1T", [128, 512], FP32, kind="ExternalInput")
    W2T_d = dt("W2T", [128, 128], FP32, kind="ExternalInput")
    W3Ts_d = dt("W3Ts", [128, 128], FP32, kind="ExternalInput")
    WinT_d = dt("WinT", [128, 512], FP32, kind="ExternalInput")
    WoutT_d = dt("WoutT", [128, 512], FP32, kind="ExternalInput")
    # per-partition vectors: [b1, b2, woutb, b3s, ln1w, ln1b, ln2w, ln2b]
    vecs_d = dt("vecs", [128, 8], FP32, kind="ExternalInput")
    winb_d = dt("winb", [128, 4], FP32, kind="ExternalInput")
    onesr_d = dt("ones_row", [1, 128], FP32, kind="ExternalInput")
    out_d = dt("out", [128, nodes_c], FP32, kind="ExternalOutput")

    with tile.TileContext(nc) as tc, nc.allow_low_precision(
        reason="float32r outputs are 32-bit storage (PE rounding mode only)"
    ):
        with ExitStack() as ctx:
            consts = ctx.enter_context(tc.tile_pool(name="consts", bufs=1))
            xpool = ctx.enter_context(tc.tile_pool(name="xpool", bufs=3))
            mpool = ctx.enter_context(tc.tile_pool(name="mpool", bufs=4))
            g1pool = ctx.enter_context(tc.tile_pool(name="g1pool", bufs=3))
            g2pool = ctx.enter_context(tc.tile_pool(name="g2pool", bufs=3))
            gmpool = ctx.enter_context(tc.tile_pool(name="gmpool", bufs=2))
            p2pool = ctx.enter_context(tc.tile_pool(name="p2pool", bufs=2))
            spool = ctx.enter_context(tc.tile_pool(name="spool", bufs=2))
            psum = ctx.enter_context(
                tc.tile_pool(name="psum", bufs=2, space="PSUM")
            )

            # ---- constants into SBUF ----
            W1T = consts.tile([128, 512], FP32)
            nc.sync.dma_start(out=_r(W1T[:, :]), in_=_r(W1T_d[:, :]))
            W2T = consts.tile([128, 128], FP32)
            nc.sync.dma_start(out=_r(W2T[:, :]), in_=_r(W2T_d[:, :]))
            W3Ts = consts.tile([128, 128], FP32)
            nc.sync.dma_start(out=_r(W3Ts[:, :]), in_=_r(W3Ts_d[:, :]))
            WinT = consts.tile([128, 512], FP32)
            nc.sync.dma_start(out=_r(WinT[:, :]), in_=_r(WinT_d[:, :]))
            WoutT = consts.tile([128, 512], FP32)
            nc.sync.dma_start(out=_r(WoutT[:, :]), in_=_r(WoutT_d[:, :]))
            vecs = consts.tile([128, 8], FP32)
            nc.sync.dma_start(out=vecs, in_=vecs_d[:, :])
            winb = consts.tile([128, 4], FP32)
            nc.sync.dma_start(out=winb, in_=winb_d[:, :])
            hVT = consts.tile([128, nodes_c], FP32)
            nc.sync.dma_start(out=_r(hVT[:, :]), in_=_r(hVT_d[:, :]))
            mV = consts.tile([1, nodes_c], FP32)
            nc.sync.dma_start(out=_r(mV[:, :]), in_=_r(mV_d[:, :]))
            ones = consts.tile([128, 1], FP32)
            nc.vector.memset(ones, 1.0)
            zero128 = consts.tile([128, 1], FP32)
            nc.vector.memset(zero128, 0.0)
            eps1 = consts.tile([1, 1], FP32)
            nc.vector.memset(eps1, EPS)
            zero1 = consts.tile([1, 1], FP32)
            nc.vector.memset(zero1, 0.0)
            ones_row = consts.tile([1, 128], FP32)
            nc.sync.dma_start(out=_r(ones_row[:, :]), in_=_r(onesr_d[:, :]))
            b1 = vecs[:, 0:1]
            b2 = vecs[:, 1:2]
            woutb = vecs[:, 2:3]
            b3s = vecs[:, 3:4]
            ln1w, ln1b = vecs[:, 4:5], vecs[:, 5:6]
            ln2w, ln2b = vecs[:, 6:7], vecs[:, 7:8]

            strips = [
                consts.tile([128, n_half], FP32, name=f"strip{s}", tag=f"strip{s}")
                for s in range(n_strips)
            ]
            msums = (
                [
                    consts.tile([1, n_half], FP32, name=f"msum{s}", tag=f"msum{s}")
                    for s in range(n_strips)
                ]
                if b3_nonzero
                else None
            )

            def one_pass():
                # ---- phase 1: message MLP + masked K-sum, per 16-node tile ----
                for t in range(n_tiles):
                    s_idx, s_col = divmod(t, tiles_per_strip)
                    col0 = s_col * TILE_N

                    xt = xpool.tile([128, 3, ROWS], FP32)
                    for c2 in range(3):
                        nc.sync.dma_start(out=_r(xt[:, c2, :]), in_=_r(hE_t[t, c2]))
                    mt = mpool.tile([1, ROWS], FP32)
                    nc.sync.dma_start(out=_r(mt[:, :]), in_=_r(mA_d[t : t + 1, :]))

                    m1h = []
                    for h in range(2):
                        m1 = psum.tile([128, HALF], FP32, tag="m1", bufs=3)
                        node0 = t * TILE_N + h * (TILE_N // 2)
                        hv_rhs = (
                            hVT[:, node0 : node0 + TILE_N // 2]
                            .unsqueeze(-1)
                            .broadcast_to([128, TILE_N // 2, K])
                        )
                        nc.tensor.matmul(
                            m1, _r(W1T[:, 0:128]), _r(hv_rhs), start=True, stop=False
                        )
                        for c2 in range(3):
                            nc.tensor.matmul(
                                m1,
                                _r(W1T[:, (c2 + 1) * 128 : (c2 + 2) * 128]),
                                _r(xt[:, c2, h * HALF : (h + 1) * HALF]),
                                start=False,
                                stop=(c2 == 2),
                            )
                        m1h.append(m1)

                    g1 = g1pool.tile([128, ROWS], FP32)
                    for h in range(2):
                        nc.scalar.activation(
                            _r(g1[:, h * HALF : (h + 1) * HALF]),
                            m1h[h],
                            AF.Gelu,
                            bias=b1,
                        )

                    m2h = []
                    for h in range(2):
                        m2 = psum.tile([128, HALF], FP32, tag="m2", bufs=3)
                        nc.tensor.matmul(
                            m2,
                            _r(W2T[:, :]),
                            _r(g1[:, h * HALF : (h + 1) * HALF]),
                            start=True,
                            stop=True,
                        )
                        m2h.append(m2)

                    g2 = g2pool.tile([128, ROWS], FP32)
                    for h in range(2):
                        nc.scalar.activation(
                            g2[:, h * HALF : (h + 1) * HALF], m2h[h], AF.Gelu, bias=b2
                        )

                    gm = gmpool.tile([128, ROWS], FP32)
                    for h in range(2):
                        mm = psum.tile([128, HALF], FP32, tag="mm", bufs=2)
                        nc.tensor.matmul(
                            mm,
                            _r(ones_row[:, :]),
                            _r(mt[:, h * HALF : (h + 1) * HALF]),
                            start=True,
                            stop=True,
                        )
                        nc.vector.tensor_mul(
                            gm[:, h * HALF : (h + 1) * HALF],
                            g2[:, h * HALF : (h + 1) * HALF],
                            mm,
                        )
                    nc.vector.tensor_reduce(
                        out=_r(strips[s_idx][:, col0 : col0 + TILE_N]),
                        in_=gm[:, :].rearrange("p (n k) -> p n k", k=K),
                        axis=mybir.AxisListType.X,
                        op=OP.add,
                    )
                    if msums is not None:
                        nc.vector.tensor_reduce(
                            out=_r(msums[s_idx][:, col0 : col0 + TILE_N]),
                            in_=mt[:, :].rearrange("p (n k) -> p n k", k=K),
                            axis=mybir.AxisListType.X,
                            op=OP.add,
                        )

                # ---- phase 2: W3 + residual + LN1 + FFN + LN2 + mask ----
                def layer_norm(x, w_ap, b_ap, width, out_r=False):
                    s1 = psum.tile([1, width], FP32, tag="m2", bufs=3)
                    nc.tensor.matmul(s1, ones, x, start=True, stop=True)
                    mu = spool.tile([1, width], FP32, tag="mu")
                    nc.scalar.activation(_r(mu[:, :]), s1, AF.Copy, scale=1.0 / 128.0)
                    mup = psum.tile([128, width], FP32, tag="mm", bufs=2)
                    nc.tensor.matmul(mup, _r(ones_row[:, :]), _r(mu[:, :]), start=True, stop=True)
                    z = p2pool.tile([128, width], FP32, tag="z")
                    nc.vector.tensor_sub(z, x, mup)
                    zsq = p2pool.tile([128, width], FP32, tag="zsq")
                    nc.scalar.activation(zsq, z, AF.Square, bias=zero128)
                    s2 = psum.tile([1, width], FP32, tag="m2", bufs=3)
                    nc.tensor.matmul(s2, ones, zsq, start=True, stop=True)
                    lnv = spool.tile([1, width], FP32, tag="lnv")
                    nc.scalar.activation(lnv, s2, AF.Ln, scale=1.0 / 128.0, bias=eps1)
                    rstd = spool.tile([1, width], FP32, tag="rstd")
                    nc.scalar.activation(_r(rstd[:, :]), lnv, AF.Exp, scale=-0.5, bias=zero1)
                    rp = psum.tile([128, width], FP32, tag="mm", bufs=2)
                    nc.tensor.matmul(rp, _r(ones_row[:, :]), _r(rstd[:, :]), start=True, stop=True)
                    zn = p2pool.tile([128, width], FP32, tag="zn")
                    nc.vector.tensor_mul(zn, z, rp)
                    o = p2pool.tile([128, width], FP32, tag="lnout")
                    nc.vector.tensor_scalar(
                    _r(o[:, :]) if out_r else o, zn, w_ap, b_ap, op0=OP.mult, op1=OP.add
                )
                    return o

                for s in range(n_strips):
                    sl = slice(s * n_half, (s + 1) * n_half)
                    dh = psum.tile([128, n_half], FP32, tag="m1", bufs=3)
                    nc.tensor.matmul(dh, _r(W3Ts[:, :]), _r(strips[s][:, :]), start=True, stop=True)
                    u = p2pool.tile([128, n_half], FP32, tag="u")
                    if msums is not None:
                        msp = psum.tile([128, n_half], FP32, tag="mm", bufs=2)
                        nc.tensor.matmul(msp, ones_row, msums[s], start=True, stop=True)
                        bm = p2pool.tile([128, n_half], FP32, tag="bm")
                        nc.vector.tensor_scalar(bm, msp, b3s, None, op0=OP.mult)
                        nc.vector.tensor_add(u, dh, bm)
                        nc.vector.tensor_add(u, u, hVT[:, sl])
                    else:
                        nc.vector.tensor_add(u, dh, hVT[:, sl])

                    h1 = layer_norm(u, ln1w, ln1b, n_half, out_r=True)

                    aT = p2pool.tile([128, 4, n_half], FP32, tag="aT")
                    for c in range(4):
                        ac = psum.tile([128, n_half], FP32, tag="m1", bufs=3)
                        nc.tensor.matmul(
                            ac,
                            _r(WinT[:, c * 128 : (c + 1) * 128]),
                            _r(h1[:, :]),
                            start=True,
                            stop=True,
                        )
                        nc.scalar.activation(
                            _r(aT[:, c, :]), ac, AF.Gelu, bias=winb[:, c : c + 1]
                        )
                    dh2 = psum.tile([128, n_half], FP32, tag="m2", bufs=3)
                    for c in range(4):
                        nc.tensor.matmul(
                            dh2,
                            _r(WoutT[:, c * 128 : (c + 1) * 128]),
                            _r(aT[:, c, :]),
                            start=(c == 0),
                            stop=(c == 3),
                        )
                    v = p2pool.tile([128, n_half], FP32, tag="v")
                    nc.vector.scalar_tensor_tensor(
                        v, in0=dh2, scalar=woutb, in1=h1, op0=OP.add, op1=OP.add
                    )
                    h2 = layer_norm(v, ln2w, ln2b, n_half)
                    mvp = psum.tile([128, n_half], FP32, tag="mm", bufs=2)
                    nc.tensor.matmul(mvp, _r(ones_row[:, :]), _r(mV[:, sl]), start=True, stop=True)
                    ot = p2pool.tile([128, n_half], FP32, tag="ot")
                    nc.vector.tensor_mul(ot, h2, mvp)
                    nc.sync.dma_start(out=out_d[:, sl], in_=ot)


            # reps>1 re-runs the whole computation for benchmarking
            for _rep in range(reps):
                one_pass()

    if split_waits:
        # required for walrus codegen; the CoreSim path must skip it
        _split_sync_waits(nc)
    return nc


def _chunkT(w):
    """[O, 4*128] row-major -> [128, 4*128] packed so cols [c*128:(c+1)*128]
    are the lhsT of chunk c (i.e. pack[p, c*128+m] = w[m, c*128+p])."""
    o = w.shape[0]
    return (
        np.ascontiguousarray(w.T.reshape(4, 128, o).transpose(1, 0, 2))
        .reshape(128, 4 * o)
        .astype(np.float32)
    )


def pack_core_inputs(hE_c, hV_c, mA_c, mV_c):
    """Per-core tensors -> device layouts (pure layout, no arithmetic)."""
    nodes_c = hV_c.shape[0]
    n_tiles = nodes_c // TILE_N
    hE_t = np.ascontiguousarray(
        hE_c.reshape(n_tiles, TILE_N, K, 3, 128).transpose(0, 3, 4, 1, 2)
    ).reshape(n_tiles, 3, 128, ROWS)
    hVT = np.ascontiguousarray(hV_c.T)
    mA = np.ascontiguousarray(mA_c.reshape(n_tiles, ROWS))
    mV = np.ascontiguousarray(mV_c.reshape(1, nodes_c))
    return {"hE_t": hE_t, "hVT": hVT, "mask_a": mA, "mask_v": mV}


def pack_weights(
    W1_w, W1_b, W2_w, W2_b, W3_w, W3_b, ln1_w, ln1_b, Win_w, Win_b,
    Wout_w, Wout_b, ln2_w, ln2_b,
):
    f32 = lambda a: np.asarray(a, np.float32)
    W1T = _chunkT(f32(W1_w))                      # [128, 512] (hv, e0, e1, e2)
    W2T = np.ascontiguousarray(f32(W2_w).T)
    W3Ts = np.ascontiguousarray(f32(W3_w).T / SCALE)
    WinT = np.ascontiguousarray(f32(Win_w).T)     # [128, 512]
    WoutT = (
        np.ascontiguousarray(f32(Wout_w).T.reshape(4, 128, 128).transpose(1, 0, 2))
        .reshape(128, 512)
    )
    vecs = np.zeros((128, 8), np.float32)
    vecs[:, 0] = f32(W1_b)
    vecs[:, 1] = f32(W2_b)
    vecs[:, 2] = f32(Wout_b)
    vecs[:, 3] = f32(W3_b) / SCALE
    vecs[:, 4] = f32(ln1_w)
    vecs[:, 5] = f32(ln1_b)
    vecs[:, 6] = f32(ln2_w)
    vecs[:, 7] = f32(ln2_b)
    winb = np.ascontiguousarray(f32(Win_b).reshape(4, 128).T)
    return {
        "W1T": W1T, "W2T": W2T, "W3Ts": W3Ts, "WinT": WinT, "WoutT": WoutT,
        "vecs": vecs, "winb": winb,
        "ones_row": np.ones((1, 128), np.float32),
    }, bool(np.any(np.asarray(W3_b)))


_PROGRAM_CACHE = {}


def prepare_run(
    h_V, h_E, mask_V, mask_attend,
    W1_w, W1_b, W2_w, W2_b, W3_w, W3_b,
    ln1_w, ln1_b, Win_w, Win_b, Wout_w, Wout_b, ln2_w, ln2_b,
):
    hV = np.asarray(h_V, np.float32).reshape(NODES, H)
    hE = np.asarray(h_E, np.float32).reshape(NODES, K, HIN)
    mA = np.asarray(mask_attend, np.float32).reshape(NODES, K)
    mV = np.asarray(mask_V, np.float32).reshape(NODES)

    wmap, b3_nonzero = pack_weights(
        W1_w, W1_b, W2_w, W2_b, W3_w, W3_b, ln1_w, ln1_b,
        Win_w, Win_b, Wout_w, Wout_b, ln2_w, ln2_b,
    )

    key = (NODES_C, N_CORES, b3_nonzero)
    nc = _PROGRAM_CACHE.get(key)
    if nc is None:
        nc = build_program(b3_nonzero=b3_nonzero)
        _PROGRAM_CACHE[key] = nc

    in_maps = []
    for c in range(N_CORES):
        sl = slice(c * NODES_C, (c + 1) * NODES_C)
        m = pack_core_inputs(hE[sl], hV[sl], mA[sl], mV[sl])
        m.update(wmap)
        in_maps.append(m)
    return nc, in_maps


def kernel(
    h_V, h_E, mask_V, mask_attend,
    W1_w, W1_b, W2_w, W2_b, W3_w, W3_b,
    ln1_w, ln1_b, Win_w, Win_b, Wout_w, Wout_b, ln2_w, ln2_b,
    *, _trace=False, _trace_cores=None,
):
    nc, in_maps = prepare_run(
        h_V, h_E, mask_V, mask_attend,
        W1_w, W1_b, W2_w, W2_b, W3_w, W3_b,
        ln1_w, ln1_b, Win_w, Win_b, Wout_w, Wout_b, ln2_w, ln2_b,
    )

    last_err = None
    for _attempt in range(3):
        try:
            res = run_bass_kernel_spmd(
                nc,
                in_maps,
                core_ids=list(range(N_CORES)),
                trace=_trace,
                trace_cores=_trace_cores,
            )
            break
        except Exception as e:  # wedged device: retry
            last_err = e
    else:
        raise last_err

    out = np.concatenate([r["out"].T for r in res.results], axis=0)
    result = out.reshape(B, N, H).astype(np.float32)
    if _trace:
        return result, res
    return result



# revision 20
# speedup vs baseline: 1.0714x; 1.0714x over previous
"""Trainium2 Bass/Tile kernel for nn_DecLayer (GNN message-passing decoder layer).

Math (per node n over K=48 neighbors):
    h_EV    = concat([h_V[n] bcast over K, h_E[n]])            [K, 512]
    m3      = gelu(gelu(h_EV @ W1.T) @ W2.T)  (W3 deferred)    [K, 128]
    dh      = (sum_k mask[k] * m3[k]) @ (W3.T/30)              [128]
    h       = LN1(h_V + dh); h = LN2(h + FFN(h)); out = mask_V * h
SCALE/EPS per the reference; W3 is applied after the K-sum (linearity).

Strategy: data-parallel over the B*N = 8192 nodes across 8 cores (1024 each).
The host pre-packs h_E *block-transposed* (feature-major) so every device DMA
is a large contiguous read with features on SBUF partitions. h_E for GROUP=2
compute tiles (32 nodes, 2.36 MB) is fetched by ONE dma_start — the DMA issue
path (SEQ + HWDGE fixed cost ~650 ns per instruction) otherwise serializes
ahead of the ~360 GB/s data movement. All matmuls keep features on partitions
(weights stationary), so biases are per-partition ACT bias vectors. The masked
K-sum is a DVE grouped reduce. When mask_attend/mask_V are all-ones (the
common case) a variant program skips the mask DMAs/multiplies entirely; the
general masked program is kept for arbitrary inputs. The W3 matmul, both
LayerNorms and the FFN run over [128, nodes] at the end (tiny vs. phase 1).
"""

import numpy as np
from contextlib import ExitStack

import concourse.bass as bass
import concourse.tile as tile
from concourse import mybir
from concourse.bass_utils import run_bass_kernel_spmd
from concourse.vector_clock import ScopedClock

# Problem shapes (fixed by the harness).
B, N, K = 4, 2048, 48
H, HIN, DFF = 128, 384, 512
SCALE, EPS = 30.0, 1e-5
N_CORES = 8
NODES = B * N
NODES_C = NODES // N_CORES   # 1024 nodes per core
TILE_N = 16                  # nodes per phase-1 compute tile
ROWS = TILE_N * K            # 768 (rows per tile)
HALF = ROWS // 2             # 384 (fp32 matmul free-dim limit is 512)
GROUP = 2                    # compute tiles fetched per dma_start
GROWS = GROUP * 3 * ROWS     # free-dim elems per partition per group DMA
FP32 = mybir.dt.float32
FP32R = mybir.dt.float32r


def _r(ap):
    """fp32 -> float32r view: PE runs float32r at 1 cycle/row (vs 4 for fp32)
    when the moving dim is >=256, at slightly reduced mantissa precision."""
    return ap.bitcast(FP32R)
AF = mybir.ActivationFunctionType
OP = mybir.AluOpType

_MAX_DRAIN_WAITS = 1  # walrus CTRL codegen accepts only 1 sync wait per Drain


def _patch_tile_drain():
    """Split the Tile tail-drain's sem waits across several Drain insts.

    The stock `_drain_and_barrier` puts every outstanding sem wait on one
    Drain; walrus's CTRL lowering in this toolchain rejects >1 wait.
    """
    if getattr(tile.TileContext, "_drain_patched", False):
        return

    def _drain_and_barrier(self, tick_clock, wait_clock):
        nc = self.nc
        drain_inst = nc.sync.drain()
        wait_clock.add_sem_waits(
            drain_inst.ins, ScopedClock({None: tick_clock.global_clock})
        )
        si = drain_inst.ins.sync_info
        waits = list(si.on_wait or []) if si is not None else []
        if len(waits) > _MAX_DRAIN_WAITS:
            si.on_wait = waits[:_MAX_DRAIN_WAITS]
            rest = waits[_MAX_DRAIN_WAITS:]
            while rest:
                d2 = nc.sync.drain()
                d2.ins.sync_info = mybir.SyncInfo(
                    on_wait=rest[:_MAX_DRAIN_WAITS], on_update=[]
                )
                rest = rest[_MAX_DRAIN_WAITS:]
        nc.all_engine_barrier()
        assert self.sems is not None
        popped = nc._tile_sem_poison_stack.pop()
        assert popped is self._sem_poison
        nc.clear_and_free_semaphores(list(self.sems.allocated().values()))
        nc.all_engine_barrier()

    tile.TileContext._drain_and_barrier = _drain_and_barrier
    tile.TileContext._drain_patched = True


def _split_sync_waits(nc, max_waits=_MAX_DRAIN_WAITS):
    """Hoist excess per-instruction sem waits onto same-engine NOPs.

    This walrus build rejects >1 sync wait on any instruction; a NOP that
    waits immediately before the real instruction is equivalent (same
    engine, program order).
    """
    for f in nc.m.functions:
        for b in f.blocks:
            new_insts = []
            for inst in b.instructions:
                si = getattr(inst, "sync_info", None)
                waits = list(si.on_wait) if si is not None and si.on_wait else []
                if len(waits) > max_waits:
                    head, keep = waits[:-max_waits], waits[-max_waits:]
                    for i in range(0, len(head), max_waits):
                        new_insts.append(
                            mybir.InstNoOp(
                                name=f"{inst.name}w{i}",
                                engine=inst.engine,
                                sync_info=mybir.SyncInfo(
                                    on_wait=head[i : i + max_waits], on_update=[]
                                ),
                                bass_nofuse=True,
                            )
                        )
                    si.on_wait = keep
                new_insts.append(inst)
            b.instructions[:] = new_insts


def build_program(nodes_c=NODES_C, num_devices=N_CORES, b3_nonzero=False,
                  mask_ones=False, split_waits=True, reps=1):
    """Build the per-core Bass program (SPMD: same program, per-core data)."""
    _patch_tile_drain()
    n_tiles = nodes_c // TILE_N
    n_groups = n_tiles // GROUP
    # Phase-2 strips: a wide strip early (matmul efficiency), narrow strips
    # late so the final strip's serial LN/FFN chain — the only part that can't
    # overlap phase 1 — is short.
    if nodes_c == 1024:
        STRIPS = [(0, 512), (512, 256), (768, 128), (896, 128)]
    else:
        STRIPS = [(o, min(512, nodes_c - o)) for o in range(0, nodes_c, 512)]
    n_strips = len(STRIPS)

    nc = bass.Bass(
        "TRN2",
        target_bir_lowering=False,
        debug=False,
        enable_asserts=False,
        num_devices=num_devices,
    )

    dt = nc.dram_tensor
    hE_t = dt("hE_t", [n_groups, 128, GROWS], FP32, kind="ExternalInput")
    hVT_d = dt("hVT", [128, nodes_c], FP32, kind="ExternalInput")
    W1T_d = dt("W1T", [128, 512], FP32, kind="ExternalInput")
    W2T_d = dt("W2T", [128, 128], FP32, kind="ExternalInput")
    W3Ts_d = dt("W3Ts", [128, 128], FP32, kind="ExternalInput")
    WinT_d = dt("WinT", [128, 512], FP32, kind="ExternalInput")
    WoutT_d = dt("WoutT", [128, 512], FP32, kind="ExternalInput")
    # per-partition vectors: [b1, b2, woutb, b3s, ln1w, ln1b, ln2w, ln2b]
    vecs_d = dt("vecs", [128, 8], FP32, kind="ExternalInput")
    winb_d = dt("winb", [128, 4], FP32, kind="ExternalInput")
    onesr_d = dt("ones_row", [1, 128], FP32, kind="ExternalInput")
    # LN weight rows for fused broadcast matmuls (bias applied per-partition)
    ln1rows_d = dt("ln1rows", [1, 128], FP32, kind="ExternalInput")
    ln2rows_d = dt("ln2rows", [1, 128], FP32, kind="ExternalInput")
    if not mask_ones:
        mA_d = dt("mask_a", [n_groups, GROUP * ROWS], FP32, kind="ExternalInput")
        mV_d = dt("mask_v", [1, nodes_c], FP32, kind="ExternalInput")
    out_d = dt("out", [128, nodes_c], FP32, kind="ExternalOutput")

    with tile.TileContext(nc) as tc, nc.allow_low_precision(
        reason="float32r outputs are 32-bit storage (PE rounding mode only)"
    ):
        with ExitStack() as ctx:
            consts = ctx.enter_context(tc.tile_pool(name="consts", bufs=1))
            xpool = ctx.enter_context(tc.tile_pool(name="xpool", bufs=4))
            g1pool = ctx.enter_context(tc.tile_pool(name="g1pool", bufs=3))
            g2pool = ctx.enter_context(tc.tile_pool(name="g2pool", bufs=3))
            p2pool = ctx.enter_context(tc.tile_pool(name="p2pool", bufs=2))
            spool = ctx.enter_context(tc.tile_pool(name="spool", bufs=2))
            psum = ctx.enter_context(
                tc.tile_pool(name="psum", bufs=2, space="PSUM")
            )
            if not mask_ones:
                mpool = ctx.enter_context(tc.tile_pool(name="mpool", bufs=3))
                gmpool = ctx.enter_context(tc.tile_pool(name="gmpool", bufs=2))

            # ---- first h_E group DMA ahead of consts: its long data phase
            # covers the HWDGE issue latency of the small const DMAs ----
            xt0 = xpool.tile([128, GROUP, 3, ROWS], FP32, name="xt0", tag="xt")
            nc.sync.dma_start(
                out=_r(xt0[:, :].rearrange("p g c r -> p (g c r)")),
                in_=_r(hE_t[0]),
            )

            # ---- constants into SBUF ----
            W1T = consts.tile([128, 512], FP32)
            nc.sync.dma_start(out=_r(W1T[:, :]), in_=_r(W1T_d[:, :]))
            hVT = consts.tile([128, nodes_c], FP32)
            nc.sync.dma_start(out=_r(hVT[:, :]), in_=_r(hVT_d[:, :]))
            vecs = consts.tile([128, 8], FP32)
            nc.sync.dma_start(out=vecs, in_=vecs_d[:, :])
            W2T = consts.tile([128, 128], FP32)
            nc.sync.dma_start(out=_r(W2T[:, :]), in_=_r(W2T_d[:, :]))
            winb = consts.tile([128, 4], FP32)
            nc.sync.dma_start(out=winb, in_=winb_d[:, :])
            # phase-2-only consts are DMA'd later (after group 2's fetch) so
            # their issue latency hides under hE data phases
            W3Ts = consts.tile([128, 128], FP32)
            WinT = consts.tile([128, 512], FP32)
            WoutT = consts.tile([128, 512], FP32)
            ln1rows = consts.tile([1, 128], FP32)
            ln2rows = consts.tile([1, 128], FP32)

            def load_phase2_consts():
                nc.sync.dma_start(out=_r(W3Ts[:, :]), in_=_r(W3Ts_d[:, :]))
                nc.sync.dma_start(out=_r(WinT[:, :]), in_=_r(WinT_d[:, :]))
                nc.sync.dma_start(out=_r(WoutT[:, :]), in_=_r(WoutT_d[:, :]))
                nc.sync.dma_start(out=_r(ln1rows[:, :]), in_=_r(ln1rows_d[:, :]))
                nc.sync.dma_start(out=_r(ln2rows[:, :]), in_=_r(ln2rows_d[:, :]))
            if not mask_ones:
                mV = consts.tile([1, nodes_c], FP32)
                nc.sync.dma_start(out=_r(mV[:, :]), in_=_r(mV_d[:, :]))
            inv128 = consts.tile([128, 1], FP32)
            nc.vector.memset(inv128, 1.0 / 128.0)
            zero128 = consts.tile([128, 1], FP32)
            nc.vector.memset(zero128, 0.0)
            eps1 = consts.tile([1, 1], FP32)
            nc.vector.memset(eps1, EPS)
            zero1 = consts.tile([1, 1], FP32)
            nc.vector.memset(zero1, 0.0)
            ones_row = consts.tile([1, 128], FP32)
            nc.sync.dma_start(out=_r(ones_row[:, :]), in_=_r(onesr_d[:, :]))
            b1 = vecs[:, 0:1]
            b2 = vecs[:, 1:2]
            woutb = vecs[:, 2:3]
            b3s = vecs[:, 3:4]
            ln1w, ln1b = vecs[:, 4:5], vecs[:, 5:6]
            ln2w, ln2b = vecs[:, 6:7], vecs[:, 7:8]

            strips = [
                consts.tile([128, w], FP32, name=f"strip{s}", tag=f"strip{s}")
                for s, (o, w) in enumerate(STRIPS)
            ]
            msums = (
                [
                    consts.tile([1, w], FP32, name=f"msum{s}", tag=f"msum{s}")
                    for s, (o, w) in enumerate(STRIPS)
                ]
                if b3_nonzero and not mask_ones
                else None
            )

            def strip_of(t):
                node0 = t * TILE_N
                for s, (o, w) in enumerate(STRIPS):
                    if o <= node0 < o + w:
                        return s, node0 - o
                raise AssertionError(t)

            def tile_body(t, xt, i, mt=None):
                """Phase-1 compute for tile t using slice i of group buffer."""
                s_idx, col0 = strip_of(t)

                m1h = []
                for h in range(2):
                    m1 = psum.tile([128, HALF], FP32, tag="m1", bufs=3)
                    node0 = t * TILE_N + h * (TILE_N // 2)
                    hv_rhs = (
                        hVT[:, node0 : node0 + TILE_N // 2]
                        .unsqueeze(-1)
                        .broadcast_to([128, TILE_N // 2, K])
                    )
                    nc.tensor.matmul(
                        m1, _r(W1T[:, 0:128]), _r(hv_rhs), start=True, stop=False
                    )
                    for c2 in range(3):
                        nc.tensor.matmul(
                            m1,
                            _r(W1T[:, (c2 + 1) * 128 : (c2 + 2) * 128]),
                            _r(xt[:, i, c2, h * HALF : (h + 1) * HALF]),
                            start=False,
                            stop=(c2 == 2),
                        )
                    m1h.append(m1)

                g1 = g1pool.tile([128, ROWS], FP32)
                for h in range(2):
                    nc.scalar.activation(
                        _r(g1[:, h * HALF : (h + 1) * HALF]),
                        m1h[h],
                        AF.Gelu,
                        bias=b1,
                    )

                m2h = []
                for h in range(2):
                    m2 = psum.tile([128, HALF], FP32, tag="m2", bufs=3)
                    nc.tensor.matmul(
                        m2,
                        _r(W2T[:, :]),
                        _r(g1[:, h * HALF : (h + 1) * HALF]),
                        start=True,
                        stop=True,
                    )
                    m2h.append(m2)

                g2 = g2pool.tile([128, ROWS], FP32)
                for h in range(2):
                    nc.scalar.activation(
                        g2[:, h * HALF : (h + 1) * HALF], m2h[h], AF.Gelu, bias=b2
                    )

                if mt is None:
                    red_in = g2
                else:
                    gm = gmpool.tile([128, ROWS], FP32)
                    for h in range(2):
                        mm = psum.tile([128, HALF], FP32, tag="mm", bufs=2)
                        nc.tensor.matmul(
                            mm,
                            _r(ones_row[:, :]),
                            _r(mt[:, h * HALF : (h + 1) * HALF]),
                            start=True,
                            stop=True,
                        )
                        nc.vector.tensor_mul(
                            gm[:, h * HALF : (h + 1) * HALF],
                            g2[:, h * HALF : (h + 1) * HALF],
                            mm,
                        )
                    red_in = gm
                nc.vector.tensor_reduce(
                    out=_r(strips[s_idx][:, col0 : col0 + TILE_N]),
                    in_=red_in[:, :].rearrange("p (n k) -> p n k", k=K),
                    axis=mybir.AxisListType.X,
                    op=OP.add,
                )
                if msums is not None:
                    nc.vector.tensor_reduce(
                        out=_r(msums[s_idx][:, col0 : col0 + TILE_N]),
                        in_=mt[:, :].rearrange("p (n k) -> p n k", k=K),
                        axis=mybir.AxisListType.X,
                        op=OP.add,
                    )

            def ln_steps(x_fn, wrow, bcol, width, out_r=False):
                """Shortened LayerNorm: var = E[x^2] - mu^2, then
                y = x*(w*rstd) + b - w*mu*rstd, with w folded into the two
                PE broadcast matmuls (wrow = [1,128] w row) and b applied as
                a per-partition scalar. Returns (closures, out_fn); closures
                are emitted one-at-a-time so the chain can interleave with
                phase-1 work without blocking engine FIFOs."""
                st = {}

                def c1():  # mu (rowsum/128) and x^2 in parallel; ones row prep
                    x = x_fn()
                    st["s1"] = psum.tile([1, width], FP32, tag="m2", bufs=3, name=f"ln_s1_{id(st)}")
                    nc.tensor.matmul(st["s1"], inv128, x, start=True, stop=True)
                    st["zsq"] = p2pool.tile([128, width], FP32, tag="zsq", name=f"ln_zsq_{id(st)}")
                    nc.scalar.activation(st["zsq"], x, AF.Square, bias=zero128)

                def c2():  # E[x^2]; mu^2
                    st["s2"] = psum.tile([1, width], FP32, tag="m2", bufs=3, name=f"ln_s2_{id(st)}")
                    nc.tensor.matmul(st["s2"], inv128, st["zsq"], start=True, stop=True)
                    st["musq"] = spool.tile([1, width], FP32, tag="musq", name=f"ln_musq_{id(st)}")
                    nc.scalar.activation(st["musq"], st["s1"], AF.Square, bias=zero1)

                def c3():  # var+eps; 1/(var+eps); rstd = sqrt of that
                    st["var"] = spool.tile([1, width], FP32, tag="var", name=f"ln_var_{id(st)}")
                    nc.vector.scalar_tensor_tensor(
                        st["var"], in0=st["s2"], scalar=eps1, in1=st["musq"],
                        op0=OP.add, op1=OP.subtract,
                    )
                    st["ivar"] = spool.tile([1, width], FP32, tag="ivar", name=f"ln_ivar_{id(st)}")
                    nc.vector.reciprocal(st["ivar"], st["var"])
                    st["rstd"] = spool.tile([1, width], FP32, tag="rstd", name=f"ln_rstd_{id(st)}")
                    nc.scalar.activation(st["rstd"], st["ivar"], AF.Sqrt, bias=zero1)

                def c4():  # q = mu*rstd; WP = bcast(w*rstd)
                    st["q"] = spool.tile([1, width], FP32, tag="q", name=f"ln_q_{id(st)}")
                    nc.vector.tensor_mul(st["q"], st["s1"], st["rstd"])
                    st["WP"] = psum.tile([128, width], FP32, tag="mm", bufs=2, name=f"ln_wp_{id(st)}")
                    nc.tensor.matmul(st["WP"], wrow, st["rstd"][:, :],
                                     start=True, stop=True)

                def c5():  # QP = bcast(w*mu*rstd); z1 = x*(w*rstd)
                    st["QP"] = psum.tile([128, width], FP32, tag="mm", bufs=2, name=f"ln_qp_{id(st)}")
                    nc.tensor.matmul(st["QP"], wrow, st["q"][:, :],
                                     start=True, stop=True)
                    st["z1"] = p2pool.tile([128, width], FP32, tag="z1", name=f"ln_z1_{id(st)}")
                    nc.vector.tensor_mul(st["z1"], x_fn(), st["WP"])

                def c6():  # o = (z1 + b) - w*mu*rstd
                    o = p2pool.tile([128, width], FP32, tag="lnout", name=f"ln_o_{id(st)}")
                    nc.vector.scalar_tensor_tensor(
                        _r(o[:, :]) if out_r else o, in0=st["z1"], scalar=bcol,
                        in1=st["QP"], op0=OP.add, op1=OP.subtract,
                    )
                    st["o"] = o

                return [c1, c2, c3, c4, c5, c6], lambda: st["o"]

            def phase2_steps(s):
                """Closures for strip s: W3 + residual + LN1 + FFN + LN2 + out."""
                o, width = STRIPS[s]
                sl = slice(o, o + width)
                st = {}
                steps = []

                def w3():
                    dh = psum.tile([128, width], FP32, tag="m1", bufs=3)
                    nc.tensor.matmul(dh, _r(W3Ts[:, :]), _r(strips[s][:, :]),
                                     start=True, stop=True)
                    st["dh"] = dh
                steps.append(w3)

                def resid():
                    u = p2pool.tile([128, width], FP32, tag="u")
                    if msums is not None:
                        msp = psum.tile([128, width], FP32, tag="mm", bufs=2)
                        nc.tensor.matmul(msp, ones_row, msums[s], start=True, stop=True)
                        bm = p2pool.tile([128, width], FP32, tag="bm")
                        nc.vector.tensor_scalar(bm, msp, b3s, None, op0=OP.mult)
                        nc.vector.tensor_add(u, st["dh"], bm)
                        nc.vector.tensor_add(u, u, hVT[:, sl])
                    else:
                        nc.vector.tensor_add(u, st["dh"], hVT[:, sl])
                    st["u"] = u
                steps.append(resid)

                ln1, h1_fn = ln_steps(lambda: st["u"], ln1rows[0:1, :], ln1b, width, out_r=True)
                steps.extend(ln1)

                def ffn_ac(c):
                    def f():
                        if c == 0:
                            st["aT"] = p2pool.tile([128, 4, width], FP32, tag="aT", name=f"p2_aT_{s}")
                        ac = psum.tile([128, width], FP32, tag="m1", bufs=3)
                        nc.tensor.matmul(
                            ac, _r(WinT[:, c * 128 : (c + 1) * 128]),
                            _r(h1_fn()[:, :]), start=True, stop=True,
                        )
                        nc.scalar.activation(
                            _r(st["aT"][:, c, :]), ac, AF.Gelu,
                            bias=winb[:, c : c + 1],
                        )
                    return f
                for c in range(4):
                    steps.append(ffn_ac(c))

                def ffn_out():
                    dh2 = psum.tile([128, width], FP32, tag="m2", bufs=3)
                    for c in range(4):
                        nc.tensor.matmul(
                            dh2, _r(WoutT[:, c * 128 : (c + 1) * 128]),
                            _r(st["aT"][:, c, :]), start=(c == 0), stop=(c == 3),
                        )
                    st["dh2"] = dh2
                steps.append(ffn_out)

                def vres():
                    v = p2pool.tile([128, width], FP32, tag="v")
                    nc.vector.scalar_tensor_tensor(
                        v, in0=st["dh2"], scalar=woutb, in1=h1_fn(), op0=OP.add,
                        op1=OP.add,
                    )
                    st["v"] = v
                steps.append(vres)

                ln2, h2_fn = ln_steps(lambda: st["v"], ln2rows[0:1, :], ln2b, width)
                steps.extend(ln2)

                def out_step():
                    h2 = h2_fn()
                    if mask_ones:
                        nc.gpsimd.dma_start(out=out_d[:, sl], in_=h2)
                    else:
                        mvp = psum.tile([128, width], FP32, tag="mm", bufs=2)
                        nc.tensor.matmul(mvp, _r(ones_row[:, :]), _r(mV[:, sl]),
                                         start=True, stop=True)
                        ot = p2pool.tile([128, width], FP32, tag="ot")
                        nc.vector.tensor_mul(ot, h2, mvp)
                        nc.gpsimd.dma_start(out=out_d[:, sl], in_=ot)
                steps.append(out_step)
                return steps

            # strip boundaries in tile units: emit strip s's phase-2 closures
            # once its last tile's body has been emitted
            strip_end_tile = {
                (o + w) // TILE_N - 1: s for s, (o, w) in enumerate(STRIPS)
            }

            def one_pass(first_xt=None):
                pending = []
                for g in range(n_groups):
                    if g == 0 and first_xt is not None:
                        xt = first_xt
                    else:
                        xt = xpool.tile(
                            [128, GROUP, 3, ROWS], FP32, tag="xt",
                            name=f"xt_{g}",
                        )
                        if g == n_groups - 1:
                            # split the final group's fetch per tile so the
                            # last tiles' compute starts one tile earlier,
                            # shortening the post-DMA drain
                            for i in range(GROUP):
                                nc.sync.dma_start(
                                    out=_r(xt[:, i].rearrange("p c r -> p (c r)")),
                                    in_=_r(hE_t[g, :, i * 3 * ROWS : (i + 1) * 3 * ROWS]),
                                )
                        else:
                            nc.sync.dma_start(
                                out=_r(xt[:, :].rearrange("p g c r -> p (g c r)")),
                                in_=_r(hE_t[g]),
                            )
                    if mask_ones:
                        mts = [None] * GROUP
                    else:
                        mg = mpool.tile([1, GROUP * ROWS], FP32)
                        nc.sync.dma_start(out=_r(mg[:, :]), in_=_r(mA_d[g : g + 1, :]))
                        mts = [mg[:, i * ROWS : (i + 1) * ROWS] for i in range(GROUP)]
                    if g == 2 and first_xt is not None:
                        load_phase2_consts()
                    for i in range(GROUP):
                        t = g * GROUP + i
                        tile_body(t, xt, i, mts[i])
                        s_done = strip_end_tile.get(t)
                        if s_done is not None and s_done < n_strips - 1:
                            pending.extend(phase2_steps(s_done))
                        for _ in range(3):
                            if pending:
                                pending.pop(0)()
                # drain leftovers + the final strip's chain as the tail
                for step in pending:
                    step()
                for step in phase2_steps(n_strips - 1):
                    step()

            # reps>1 re-runs the whole computation for benchmarking
            for _rep in range(reps):
                one_pass(first_xt=xt0 if _rep == 0 else None)

    if split_waits:
        # required for walrus codegen; the CoreSim path must skip it
        _split_sync_waits(nc)
    return nc


def _chunkT(w):
    """[O, 4*128] row-major -> [128, 4*128] packed so cols [c*128:(c+1)*128]
    are the lhsT of chunk c (i.e. pack[p, c*128+m] = w[m, c*128+p])."""
    o = w.shape[0]
    return (
        np.ascontiguousarray(w.T.reshape(4, 128, o).transpose(1, 0, 2))
        .reshape(128, 4 * o)
        .astype(np.float32)
    )


def pack_core_inputs(hE_c, hV_c, mA_c, mV_c, mask_ones):
    """Per-core tensors -> device layouts (pure layout, no arithmetic)."""
    nodes_c = hV_c.shape[0]
    n_tiles = nodes_c // TILE_N
    n_groups = n_tiles // GROUP
    # [groups, 128-feat, tile-in-group, c2, TILE_N*K]
    hE_t = np.ascontiguousarray(
        hE_c.reshape(n_groups, GROUP, TILE_N, K, 3, 128)
        .transpose(0, 5, 1, 4, 2, 3)
    ).reshape(n_groups, 128, GROWS)
    hVT = np.ascontiguousarray(hV_c.T)
    m = {"hE_t": hE_t, "hVT": hVT}
    if not mask_ones:
        m["mask_a"] = np.ascontiguousarray(mA_c.reshape(n_groups, GROUP * ROWS))
        m["mask_v"] = np.ascontiguousarray(mV_c.reshape(1, nodes_c))
    return m


def pack_weights(
    W1_w, W1_b, W2_w, W2_b, W3_w, W3_b, ln1_w, ln1_b, Win_w, Win_b,
    Wout_w, Wout_b, ln2_w, ln2_b,
):
    f32 = lambda a: np.asarray(a, np.float32)
    W1T = _chunkT(f32(W1_w))                      # [128, 512] (hv, e0, e1, e2)
    W2T = np.ascontiguousarray(f32(W2_w).T)
    W3Ts = np.ascontiguousarray(f32(W3_w).T / SCALE)
    WinT = np.ascontiguousarray(f32(Win_w).T)     # [128, 512]
    WoutT = (
        np.ascontiguousarray(f32(Wout_w).T.reshape(4, 128, 128).transpose(1, 0, 2))
        .reshape(128, 512)
    )
    vecs = np.zeros((128, 8), np.float32)
    vecs[:, 0] = f32(W1_b)
    vecs[:, 1] = f32(W2_b)
    vecs[:, 2] = f32(Wout_b)
    vecs[:, 3] = f32(W3_b) / SCALE
    vecs[:, 4] = f32(ln1_w)
    vecs[:, 5] = f32(ln1_b)
    vecs[:, 6] = f32(ln2_w)
    vecs[:, 7] = f32(ln2_b)
    winb = np.ascontiguousarray(f32(Win_b).reshape(4, 128).T)
    ln1rows = np.ascontiguousarray(f32(ln1_w)[None, :])
    ln2rows = np.ascontiguousarray(f32(ln2_w)[None, :])
    return {
        "W1T": W1T, "W2T": W2T, "W3Ts": W3Ts, "WinT": WinT, "WoutT": WoutT,
        "vecs": vecs, "winb": winb, "ln1rows": ln1rows, "ln2rows": ln2rows,
        "ones_row": np.ones((1, 128), np.float32),
    }, bool(np.any(np.asarray(W3_b)))


_PROGRAM_CACHE = {}


def prepare_run(
    h_V, h_E, mask_V, mask_attend,
    W1_w, W1_b, W2_w, W2_b, W3_w, W3_b,
    ln1_w, ln1_b, Win_w, Win_b, Wout_w, Wout_b, ln2_w, ln2_b,
):
    hV = np.asarray(h_V, np.float32).reshape(NODES, H)
    hE = np.asarray(h_E, np.float32).reshape(NODES, K, HIN)
    mA = np.asarray(mask_attend, np.float32).reshape(NODES, K)
    mV = np.asarray(mask_V, np.float32).reshape(NODES)
    mask_ones = bool(np.all(mA == 1.0)) and bool(np.all(mV == 1.0))

    wmap, b3_nonzero = pack_weights(
        W1_w, W1_b, W2_w, W2_b, W3_w, W3_b, ln1_w, ln1_b,
        Win_w, Win_b, Wout_w, Wout_b, ln2_w, ln2_b,
    )

    key = (NODES_C, N_CORES, b3_nonzero, mask_ones)
    nc = _PROGRAM_CACHE.get(key)
    if nc is None:
        nc = build_program(b3_nonzero=b3_nonzero, mask_ones=mask_ones)
        _PROGRAM_CACHE[key] = nc

    in_maps = []
    for c in range(N_CORES):
        sl = slice(c * NODES_C, (c + 1) * NODES_C)
        m = pack_core_inputs(hE[sl], hV[sl], mA[sl], mV[sl], mask_ones)
        m.update(wmap)
        in_maps.append(m)
    return nc, in_maps


def kernel(
    h_V, h_E, mask_V, mask_attend,
    W1_w, W1_b, W2_w, W2_b, W3_w, W3_b,
    ln1_w, ln1_b, Win_w, Win_b, Wout_w, Wout_b, ln2_w, ln2_b,
    *, _trace=False, _trace_cores=None,
):
    nc, in_maps = prepare_run(
        h_V, h_E, mask_V, mask_attend,
        W1_w, W1_b, W2_w, W2_b, W3_w, W3_b,
        ln1_w, ln1_b, Win_w, Win_b, Wout_w, Wout_b, ln2_w, ln2_b,
    )

    last_err = None
    for _attempt in range(3):
        try:
            res = run_bass_kernel_spmd(
                nc,
                in_maps,
                core_ids=list(range(N_CORES)),
                trace=_trace,
                trace_cores=_trace_cores,
            )
            break
        except Exception as e:  # wedged device: retry
            last_err = e
    else:
        raise last_err

    out = np.concatenate([r["out"].T for r in res.results], axis=0)
    result = out.reshape(B, N, H).astype(np.float32)
    if _trace:
        return result, res
    return result
